# revision 2
# baseline (speedup 1.0000x reference)
"""Bilateral filter — flipped PE-reduction design.

Identity: out = c + Sum_d(g_d * G(D_d) * D_d) / Sum_d(g_d * G(D_d)),
where D_d = shifted(I, d) - I and G(x) = exp(-50 x^2), computed on ACT in a
single pass as Derivative_Erf(sqrt(50)*x) = (2/sqrt(pi)) exp(-50 x^2) — the
constant cancels in the ratio.

Layout: partitions = 2 pixel-chunks x 49 shifts (98). Host preps the 49-shift
im2col difference map in bf16. Device: one ACT pass (weights w), one DVE
multiply (U = w*D), then PE matmuls with w/U blocks as the STATIONARY
[98,128] and the g-weights as a tiny [98,2] moving operand, producing
[128 pixels, 2 chunks] PSUM outputs — full-partition results that DVE can
divide in place (DMA cannot read PSUM).

Sharding: pure data-parallel, one image per NeuronCore (batch 8 over 8 cores).
"""
from contextlib import ExitStack

import numpy as np
import ml_dtypes

import concourse.bass as bass
import concourse.bacc as bacc
import concourse.tile as tile
from concourse import mybir

F32 = mybir.dt.float32
BF16 = mybir.dt.bfloat16

H, W = 480, 640
NPIX = H * W                  # 307200
ALEN = 155648                 # 19 * 8192, chunk-A pixels
BLEN = NPIX - ALEN            # 151552, chunk-B pixels
P = 8192                      # pixels per chunk per macro-tile
NT = 19                       # ALEN / P exactly
NB = P // 128                 # 64 blocks per macro-tile
SQRT50 = float(np.sqrt(50.0))
N_CORES = 8
PAD = 3
K = 7


def make_dmap(img):
    """[98, ALEN] bf16 im2col difference map: row k<49 = chunk A (pixels
    0..ALEN) of shift k, row k>=49 = chunk B (pixels ALEN..NPIX), zero-padded
    to ALEN columns."""
    from numpy.lib.stride_tricks import sliding_window_view
    img = np.asarray(img, np.float32)
    Ip = np.zeros((H + 2 * PAD, W + 2 * PAD), np.float32)
    Ip[PAD:PAD + H, PAD:PAD + W] = img
    sw = sliding_window_view(Ip, (H, W))          # (7, 7, H, W)
    Dm = sw.reshape(K * K, NPIX) - img.reshape(1, NPIX)
    dm = np.zeros((98, ALEN), np.float32)
    dm[:49] = Dm[:, :ALEN]
    dm[49:, :BLEN] = Dm[:, ALEN:]
    return dm.astype(ml_dtypes.bfloat16)


def make_cimg(img):
    """[NT*128, 128] f32 center image in the divide-phase layout:
    row (t, m), col e*64+u  ->  pixel e*ALEN + t*P + 64*m + u (0 past end)."""
    flat = np.asarray(img, np.float32).reshape(-1)
    ext = np.zeros(2 * ALEN, np.float32)
    ext[:NPIX] = flat
    a = ext[:ALEN].reshape(NT, 128, 64)
    b = ext[ALEN:].reshape(NT, 128, 64)
    out = np.concatenate([a, b], axis=2)          # (NT, 128, 128)
    return np.ascontiguousarray(out.reshape(NT * 128, 128))


def make_stat(g49):
    """[98, 2] bf16 moving operand: col 0 = g over chunk-A rows, col 1 = B."""
    s = np.zeros((98, 2), np.float32)
    s[:49, 0] = g49
    s[49:, 1] = g49
    return s.astype(ml_dtypes.bfloat16)


def emit(nc, dmap_ap, cimg_ap, stat_ap, out_ap, reps=1):
    derf = mybir.ActivationFunctionType.Derivative_Erf

    with tile.TileContext(nc) as tc, ExitStack() as ctx:
        singles = ctx.enter_context(tc.tile_pool(name="singles", bufs=1))
        dpool = ctx.enter_context(tc.tile_pool(name="dpool", bufs=4))
        wpool = ctx.enter_context(tc.tile_pool(name="wpool", bufs=4))
        upool = ctx.enter_context(tc.tile_pool(name="upool", bufs=4))
        qpool = ctx.enter_context(tc.tile_pool(name="qpool", bufs=3))
        ppool = ctx.enter_context(tc.tile_pool(name="ppool", bufs=4, space="PSUM"))

        stat_t = singles.tile([98, 2], BF16, name="stat")
        nc.sync.dma_start(out=stat_t, in_=stat_ap)
        warm = singles.tile([98, 2], BF16, name="warm")
        nc.scalar.activation(out=warm, in_=stat_t, func=derf,
                             bias=0.0, scale=1.0)

        for rep in range(reps):
         for t in range(NT):
            D_t = dpool.tile([98, P], BF16, name="D")
            nc.sync.dma_start(out=D_t, in_=dmap_ap[:, t * P:(t + 1) * P])
            w_t = wpool.tile([98, P], BF16, name="w")
            nc.scalar.activation(out=w_t, in_=D_t, func=derf,
                                 bias=0.0, scale=SQRT50)
            U_t = upool.tile([98, P], BF16, name="U")
            nc.vector.tensor_tensor(out=U_t, in0=w_t, in1=D_t,
                                    op=mybir.AluOpType.mult)

            pt = ppool.tile([128, 512], F32, name="pt")
            pb = pt[:]
            wb = w_t[:]
            ub = U_t[:]
            for b in range(NB):
                # stationary cols m = pixel m*64+b of this tile (both chunks)
                wst = bass.AP(tensor=wb.tensor, offset=wb.offset + b,
                              ap=[wb.ap[0], [64, 128]])
                ust = bass.AP(tensor=ub.tensor, offset=ub.offset + b,
                              ap=[ub.ap[0], [64, 128]])
                od = bass.AP(tensor=pb.tensor, offset=pb.offset + 4 * b,
                             ap=[pb.ap[0], [1, 2]])
                on = bass.AP(tensor=pb.tensor, offset=pb.offset + 4 * b + 2,
                             ap=[pb.ap[0], [1, 2]])
                nc.tensor.matmul(out=od, lhsT=wst, rhs=stat_t[:])
                nc.tensor.matmul(out=on, lhsT=ust, rhs=stat_t[:])

            # divide phase: den at cols {4b, 4b+1}, num at {4b+2, 4b+3}
            rden = qpool.tile([128, 128], F32, name="rden")
            den_ap = bass.AP(tensor=pb.tensor, offset=pb.offset,
                             ap=[pb.ap[0], [4, 64], [1, 2]])
            num_ap = bass.AP(tensor=pb.tensor, offset=pb.offset + 2,
                             ap=[pb.ap[0], [4, 64], [1, 2]])
            nc.vector.reciprocal_approx_fast(
                out=rden[:].rearrange("p (a b) -> p a b", a=64),
                in_=den_ap)
            c_t = qpool.tile([128, 128], F32, name="c_t")
            nc.gpsimd.dma_start(out=c_t,
                                in_=cimg_ap[t * 128:(t + 1) * 128, :])
            q_t = qpool.tile([128, 128], F32, name="q_t")
            qb = q_t[:]
            # write A-pixels to cols 0..63, B to 64..127 (iter order u,e)
            q_out = bass.AP(tensor=qb.tensor, offset=qb.offset,
                            ap=[qb.ap[0], [1, 64], [64, 2]])
            nc.vector.tensor_tensor(
                out=q_out, in0=num_ap,
                in1=rden[:].rearrange("p (a b) -> p a b", a=64),
                op=mybir.AluOpType.mult)
            o_t = qpool.tile([128, 128], F32, name="o_t")
            nc.vector.tensor_tensor(out=o_t, in0=q_t, in1=c_t,
                                    op=mybir.AluOpType.add)

            ob = o_t[:]
            if t < NT - 1:
                dst = bass.AP(tensor=out_ap.tensor,
                              offset=out_ap.offset + t * P,
                              ap=[[64, 128], [ALEN, 2], [1, 64]])
                src = bass.AP(tensor=ob.tensor, offset=ob.offset,
                              ap=[ob.ap[0], [64, 2], [1, 64]])
                nc.gpsimd.dma_start(out=dst, in_=src)
            else:
                dstA = bass.AP(tensor=out_ap.tensor,
                               offset=out_ap.offset + t * P,
                               ap=[[64, 128], [1, 64]])
                nc.gpsimd.dma_start(out=dstA, in_=o_t[:, 0:64])
                # B-chunk: only pixels < BLEN exist (partitions 0..63)
                dstB = bass.AP(tensor=out_ap.tensor,
                               offset=out_ap.offset + ALEN + t * P,
                               ap=[[64, 64], [1, 64]])
                nc.gpsimd.dma_start(out=dstB, in_=o_t[0:64, 64:128])


def build_nc(reps=1):
    nc = bacc.Bacc(num_devices=N_CORES)
    dmap = nc.dram_tensor("dmap", [98, ALEN], BF16, kind="ExternalInput")
    cimg = nc.dram_tensor("cimg", [NT * 128, 128], F32, kind="ExternalInput")
    stat = nc.dram_tensor("stat", [98, 2], BF16, kind="ExternalInput")
    out = nc.dram_tensor("out", [H, W], F32, kind="ExternalOutput")
    emit(nc, dmap.ap(), cimg.ap(), stat.ap(), out.ap(), reps=reps)
    nc.finalize()
    return nc


def make_in_maps(I, g49):
    in_maps = []
    stat = make_stat(g49)
    for c in range(I.shape[0]):
        img = I[c, 0]
        in_maps.append({"dmap": make_dmap(img), "cimg": make_cimg(img),
                        "stat": stat})
    return in_maps


def kernel(I: np.ndarray, g: np.ndarray) -> np.ndarray:
    from concourse.bass_utils import run_bass_kernel_spmd

    I = np.ascontiguousarray(np.asarray(I, np.float32))
    g49 = np.asarray(g, np.float32).reshape(-1)
    nc = build_nc()
    in_maps = make_in_maps(I, g49)
    res = run_bass_kernel_spmd(nc, in_maps, core_ids=list(range(N_CORES)))
    global LAST_RESULTS
    LAST_RESULTS = res
    return np.stack([r["out"] for r in res.results], axis=0)


LAST_RESULTS = None
